# revision 22
# baseline (speedup 1.0000x reference)
"""Trainium2 Bass kernel for nn_DEC_26139170963600 (vq_codebook).

Reference computation:
  4x strided conv1d (stride 2, VALID) with LeakyReLU(0.1) between layers,
  flatten -> soft VQ assignment over 64 centers:
      d2 = ||z||^2 + ||c||^2 - 2 z.c
      q  = (1/(1+d2)) row-normalized            (alpha=1 -> exponent is 1)

Sharding: data-parallel over batch N=256 across 8 cores (32 samples/core).
Weights / centers replicated. No cross-device communication.

Per-core kernel design (all-fp8e4 conv path, ~2e-3 max rel err vs fp32):
  - Convs run as fp8 DoubleRow matmuls: each instruction contracts TWO taps
    (lhsT (128,2,128) tap-weight pair, rhs (128,2,FD) strided input pair) at
    0.5 PE cycles per output column -- 4x the bf16 rate per MAC.  Odd tap
    counts get a zero pad tap (conv1 15->16, conv3 7->8).
  - DoubleRow requires a strictly 3D moving AP, and free dims < ~120 hang the
    HW under multi-core load.  Activation tiles therefore use even-padded
    rows (h1 506, h2 248, h3 122) so conv2-4 merge all G samples of a PSUM
    group into ONE matmul: moving dim = (g,l) virtually merged, FD 506/496/
    488.  The interleaved junk columns are skipped at eviction; all junk
    bytes are memset once so the pad taps never feed NaN into the PE.
  - PSUM eviction is a single ACT op: Prelu(y + b) with alpha=0.1 (HW Prelu
    is exact; HW Lrelu is broken - 0.01x on negatives), written directly as
    fp8.  conv4 evicts Identity+bias into the z columns of the crz tile.
  - Distance phase uses a combined crz tile: per position l a 128-wide block
    [-2c_j (64) | z_n (32) | zeros (32)].  30 position-pair DoubleRow matmuls
    (lhsT = z pair, rhs = full block pair, FD=128) accumulate BOTH the cross
    term z.(-2c) (cols 0:64) and the Gram matrix z.z (cols 64:96) into one
    PSUM bank; the bank is opened by one fp32 rank-1 matmul depositing
    1+||c_j||^2 so no extra DVE add is needed.  ||z_n||^2 = Gram diagonal,
    extracted with a fused identity-mask tensor_tensor_reduce.
  - q = reciprocal(1+d2) row-normalized on DVE, DMA out as fp32.
  - PE pre-warm: dummy matmuls bridge the DMA lead-in so conv work is billed
    at the ramped 2.4 GHz p-state (PE idle resets the ramp window).

fp16 matmuls hard-fault trn2 here (NRT_EXEC_UNIT_UNRECOVERABLE) - do not use.
"""

import os
import sys

import numpy as np
import ml_dtypes

for _p in ("/opt/trn_rl_repo",):
    if _p not in sys.path and os.path.isdir(_p):
        sys.path.insert(0, _p)

import concourse.bacc as bacc  # noqa: E402
import concourse.mybir as mybir  # noqa: E402
import concourse.tile as tile  # noqa: E402
from concourse import bass_utils  # noqa: E402

FP8 = mybir.dt.float8e4
F32 = mybir.dt.float32
AF = mybir.ActivationFunctionType
OP = mybir.AluOpType
PM = mybir.MatmulPerfMode
NP8 = ml_dtypes.float8_e4m3  # maps to mybir float8e4 (max +-240)

N_CORES = 8
NS = 32          # samples per core
C = 128          # channels
KCENT = 64       # number of centers
LFIN = 59        # final length
LPAD = 60        # padded (even) final length for position-pair matmuls
NPAIR = LPAD // 2
WBLK = 128       # crz per-position block: [cr 64 | z 32 | zeros 32]

# (tap pairs, in_row_len, out_len, padded_out_row, G samples per PSUM group)
CFG = [
    (8, 1024, 505, 506, 1),   # conv1 K=15(+pad), reads x chunks
    (6, 506, 247, 248, 2),    # conv2 K=12
    (4, 248, 121, 122, 4),    # conv3 K=7(+pad)
    (2, 122, 59, WBLK, 8),    # conv4 K=4, evicts into crz z-columns
]
N_WARMUP = 24  # dummy PE matmuls bridging the DMA lead-in (p-state ramp)

_BUILT = {}


def _build_program(n_repeat=1):
    """Build + compile the per-core Bass program (same program on all cores).

    n_repeat > 1 unrolls the full per-inference body inside one NEFF
    (constants loaded once) -- only used for slope-timing experiments.
    """
    nc = bacc.Bacc("TRN2", target_bir_lowering=False, debug=False)

    x_d = nc.dram_tensor("x", (C, NS, 1024), FP8, kind="ExternalInput")
    w_d = [
        nc.dram_tensor(f"w{i+1}", (C, CFG[i][0] * 2 * C), FP8, kind="ExternalInput")
        for i in range(4)
    ]
    bp_d = nc.dram_tensor("bp", (C, 4), F32, kind="ExternalInput")
    crz_d = nc.dram_tensor("crz", (C, LPAD * WBLK), FP8, kind="ExternalInput")
    cnb_d = nc.dram_tensor("cnb", (C, WBLK), F32, kind="ExternalInput")
    id_d = nc.dram_tensor("ident", (NS, NS), F32, kind="ExternalInput")
    q_d = nc.dram_tensor("q", (NS, KCENT), F32, kind="ExternalOutput")

    with tile.TileContext(nc) as tc:
        with (
            tc.tile_pool(name="consts", bufs=1) as cpool,
            tc.tile_pool(name="xp", bufs=16) as xpool,
            tc.tile_pool(name="hp", bufs=1) as hpool,
            tc.tile_pool(name="small", bufs=1) as mpool,
            tc.tile_pool(name="psA", bufs=7, space="PSUM") as psA,
            tc.tile_pool(name="psD", bufs=1, space="PSUM") as psD,
        ):
            wt = [
                cpool.tile([C, CFG[i][0] * 2 * C], FP8, tag=f"w{i}", name=f"wt{i}")
                for i in range(4)
            ]
            bp = cpool.tile([C, 4], F32, tag="bp")
            cnb = cpool.tile([C, WBLK], F32, tag="cnb")
            ident = cpool.tile([NS, NS], F32, tag="ident")
            onescol = cpool.tile([C, NS], F32, tag="onescol")

            for _rep in range(n_repeat):
                _body_once(nc, x_d, q_d, w_d, bp_d, crz_d, cnb_d, id_d, wt, bp,
                           cnb, ident, onescol, xpool, hpool, mpool, psA, psD,
                           load_consts=(_rep == 0))

    nc.compile()
    return nc


def _body_once(nc, x_d, q_d, w_d, bp_d, crz_d, cnb_d, id_d, wt, bp, cnb,
               ident, onescol, xpool, hpool, mpool, psA, psD, load_consts=True):
    # ---- DMA rings: w1 leads the SP ring, then x chunks (0/2/4 ride the
    # otherwise-idle ACT ring so both HWDGE rings pipeline the lead-in);
    # late-needed consts trail the SP ring ----
    if load_consts:
        wsrc = mpool.tile([1, 128], FP8, tag="warm", name="warm")
        nc.gpsimd.memset(wsrc[:], 0.0)
        nc.sync.dma_start(wt[0][:], w_d[0].ap())
    xch = []
    for g in range(16):
        t = xpool.tile([C, 2 * 1024], FP8, tag="x", name=f"xch{g}")
        src = x_d.ap()[:, 2 * g : 2 * g + 2, :].rearrange("p a b -> p (a b)")
        if g in (0, 2, 4):
            nc.scalar.dma_start(t[:], src)
            if g == 0 and load_consts:
                nc.scalar.dma_start(bp[:], bp_d.ap())
        else:
            nc.sync.dma_start(t[:], src)
        xch.append(t)

    # ---- activation tiles (even-padded rows + read slack, see header) ----
    h1 = hpool.tile([C, NS * 506 + 16], FP8, tag="h1")
    h2 = hpool.tile([C, NS * 248 + 8], FP8, tag="h2")
    h3 = hpool.tile([C, NS * 122 + 8], FP8, tag="h3")
    crz = hpool.tile([C, LPAD * WBLK], FP8, tag="crz")

    # junk columns / slack the pad taps may read: must be finite (not NaN)
    h1v = h1[:, : NS * 506].rearrange("p (n l) -> p n l", l=506)
    h2v = h2[:, : NS * 248].rearrange("p (n l) -> p n l", l=248)
    h3v = h3[:, : NS * 122].rearrange("p (n l) -> p n l", l=122)
    nc.gpsimd.memset(h1v[:, :, 505:506], 0.0)
    nc.gpsimd.memset(h2v[:, :, 247:248], 0.0)
    nc.gpsimd.memset(h3v[:, :, 121:122], 0.0)
    nc.gpsimd.memset(h1[:, NS * 506 :], 0.0)
    nc.gpsimd.memset(h2[:, NS * 248 :], 0.0)
    nc.gpsimd.memset(h3[:, NS * 122 :], 0.0)

    if load_consts:
        nc.gpsimd.memset(onescol[:], 1.0)
        for i in range(1, 4):
            nc.sync.dma_start(wt[i][:], w_d[i].ap())
    # crz initial image: [-2c | 0 | 0] per position (zeros include the z
    # region, so the pad position 59 stays exactly zero)
    nc.sync.dma_start(crz[:], crz_d.ap())
    if load_consts:
        nc.sync.dma_start(cnb[:], cnb_d.ap())
        nc.sync.dma_start(ident[:], id_d.ap())

    # ---- PE pre-warm ----
    if load_consts:
        wps = psA.tile([C, 128], F32, tag="ps", name="warmps")
        for _w in range(N_WARMUP):
            nc.tensor.matmul(
                wps[:], wsrc[:], wsrc[:], start=(_w == 0), stop=(_w == N_WARMUP - 1)
            )

    # ---- conv stack: fp8 DoubleRow tap-pair matmuls, G samples merged into
    # one moving AP via the even-padded rows ----
    crzv = crz[:].rearrange("p (l w) -> p w l", w=WBLK)  # (p, col-in-block, l)
    h_in = [None, h1, h2, h3]
    h_outv = [h1v, h2v, h3v, None]
    for li, (npr, lin, lout, lrow, G) in enumerate(CFG):
        wv = wt[li][:].rearrange("p (pp two m) -> p pp two m", two=2, m=C)
        nl_in = CFG[li - 1][3] if li > 0 else 0  # input padded row length
        nlm = nl_in // 2  # merged per-sample output count
        for g0 in range(0, NS, G):
            if li == 0:
                ps = psA.tile([C, 505], F32, tag="ps")
                x3 = xch[g0 // 2][:].rearrange("p (a l) -> p a l", a=2)
                for p in range(8):
                    rhs = x3[:, g0 % 2 : g0 % 2 + 1, 2 * p : 2 * p + 2 * 505]
                    rhs = rhs.rearrange("p a (l two) -> p two (a l)", two=2)
                    nc.tensor.matmul(ps[:], wv[:, p], rhs, start=(p == 0),
                                     stop=(p == 7), perf_mode=PM.DoubleRow)
                nc.scalar.activation(h1v[:, g0 : g0 + 1, 0:505], ps[:],
                                     AF.Prelu, bias=bp[:, 0:1], scale=1.0,
                                     alpha=0.1)
                continue
            fd = G * nlm
            ps = psA.tile([C, fd], F32, tag="ps", name=f"psl{li}")
            base = g0 * nl_in
            for p in range(npr):
                rhs = h_in[li][:, base + 2 * p : base + 2 * p + 2 * fd]
                rhs = rhs.rearrange("p (l two) -> p two l", two=2)
                nc.tensor.matmul(ps[:], wv[:, p], rhs, start=(p == 0),
                                 stop=(p == npr - 1), perf_mode=PM.DoubleRow)
            ps3 = ps[:].rearrange("p (g l) -> p g l", l=nlm)
            bias = bp[:, li : li + 1]
            if li < 3:
                nc.scalar.activation(
                    h_outv[li][:, g0 : g0 + G, 0:lout], ps3[:, :, 0:lout],
                    AF.Prelu, bias=bias, scale=1.0, alpha=0.1)
            else:
                # z_n for position l lives at crz col l*128 + 64 + n
                nc.scalar.activation(
                    crzv[:, 64 + g0 : 64 + g0 + G, 0:LFIN], ps3[:, :, 0:LFIN],
                    AF.Identity, bias=bias, scale=1.0)

    # ---- distance: 30 position-pair DR matmuls (FD=128) accumulate the
    # cross term (cols 0:64) and the Gram matrix (cols 64:96) on top of the
    # fp32-deposited 1+||c||^2 row ----
    d_ps = psD.tile([NS, WBLK], F32, tag="d")
    nc.tensor.matmul(d_ps[:], onescol[:], cnb[:], start=True, stop=False,
                     skip_group_check=True)
    crzp = crz[:].rearrange("p (pp two w) -> p pp two w", two=2, w=WBLK)
    for pp in range(NPAIR):
        nc.tensor.matmul(d_ps[:], crzp[:, pp, :, 64:96], crzp[:, pp],
                         start=False, stop=(pp == NPAIR - 1),
                         perf_mode=PM.DoubleRow, skip_group_check=True)

    # ---- q = normalize(1/(1+d2)) ----
    gd = mpool.tile([NS, NS], F32, tag="gd")
    zn1 = mpool.tile([NS, 1], F32, tag="zn1")
    nc.vector.tensor_tensor_reduce(gd[:], d_ps[:, 64:96], ident[:], 1.0, 0.0,
                                   op0=OP.mult, op1=OP.add, accum_out=zn1[:])
    t1 = mpool.tile([NS, KCENT], F32, tag="t1")
    nc.vector.tensor_scalar_add(t1[:], d_ps[:, 0:KCENT], zn1[:])
    qn = mpool.tile([NS, KCENT], F32, tag="qn")
    nc.vector.reciprocal(qn[:], t1[:])
    rs = mpool.tile([NS, 1], F32, tag="rs")
    nc.vector.tensor_reduce(rs[:], qn[:], axis=mybir.AxisListType.X, op=OP.add)
    rr = mpool.tile([NS, 1], F32, tag="rr")
    nc.vector.reciprocal(rr[:], rs[:])
    nc.vector.tensor_scalar_mul(qn[:], qn[:], rr[:])
    nc.sync.dma_start(q_d.ap(), qn[:])


def _get_program(n_repeat=1):
    if n_repeat not in _BUILT:
        _BUILT[n_repeat] = _build_program(n_repeat)
    return _BUILT[n_repeat]


def _prep_inputs(x, w1, b1, w2, b2, w3, b3, w4, b4, centers):
    """Host-side prep: fp8 casts, tap-pair weight layout, per-core sharding."""
    ws = [w1, w2, w3, w4]
    bs = [b1, b2, b3, b4]
    ks = [15, 12, 7, 4]

    const_map = {}
    for i, w in enumerate(ws):
        K, npr = ks[i], CFG[i][0]
        wf = np.asarray(w, np.float32)  # (O, I, K)
        wp = np.zeros((C, npr, 2, C), np.float32)
        for p in range(npr):
            for j in range(2):
                k = 2 * p + j
                if k < K:
                    wp[:, p, j, :] = wf[:, :, k].T  # [i, o]
        const_map[f"w{i+1}"] = wp.reshape(C, npr * 2 * C).astype(NP8)

    bpm = np.zeros((C, 4), np.float32)
    for i, b in enumerate(bs):
        bpm[:, i] = np.asarray(b, np.float32)
    const_map["bp"] = bpm

    cent = np.asarray(centers, np.float32)  # (64, 7552)
    c3 = cent.reshape(KCENT, C, LFIN)
    crz = np.zeros((C, LPAD, WBLK), np.float32)
    for l in range(LFIN):
        crz[:, l, 0:KCENT] = -2.0 * c3[:, :, l].T  # [c, j]
    const_map["crz"] = crz.reshape(C, LPAD * WBLK).astype(NP8)

    cn = 1.0 + (cent.astype(np.float64) ** 2).sum(axis=1)  # (64,)
    # deposited into PSUM as ones(C,NS).T @ cnb: each of the C channels
    # contributes cn_j/C, summing to 1 + ||c_j||^2 (C is a power of 2)
    cnb = np.zeros((C, WBLK), np.float32)
    cnb[:, 0:KCENT] = (cn / C).astype(np.float32)[None, :]
    const_map["cnb"] = cnb
    const_map["ident"] = np.eye(NS, dtype=np.float32)

    xf = np.asarray(x, np.float32)
    in_maps = []
    for c in range(N_CORES):
        shard = xf[c * NS : (c + 1) * NS]  # (32, 128, 1024)
        xc = np.ascontiguousarray(shard.transpose(1, 0, 2)).astype(NP8)
        in_maps.append({"x": xc, **const_map})
    return in_maps


def _ensure_devices():
    """Absorb wedged-device attach faults with a tiny op before the real run.

    A previous process can leave a NeuronCore wedged
    (NRT_EXEC_UNIT_UNRECOVERABLE); the first attach after a wedge fails and
    triggers a reset that completes within ~60 s.
    """
    import time

    import jax
    import jax.numpy as jnp

    for attempt in range(5):
        try:
            outs = [jax.device_put(jnp.zeros((8,)), d) + 1.0 for d in jax.devices()]
            jax.block_until_ready(outs)
            return
        except Exception:  # noqa: BLE001 - device fault; wait out the reset
            if attempt == 4:
                raise
            time.sleep(60)


def run(trace=False, **inputs):
    """Run the kernel; returns (q_full, BassKernelResults).

    Retries on device-unrecoverable faults (see _ensure_devices).
    """
    import time

    _ensure_devices()
    nc = _get_program()
    in_maps = _prep_inputs(**inputs)
    last_err = None
    for attempt in range(4):
        try:
            res = bass_utils.run_bass_kernel_spmd(
                nc, in_maps, core_ids=list(range(N_CORES)), trace=trace
            )
            break
        except Exception as e:  # noqa: BLE001 - device fault, wait + retry
            last_err = e
            msg = str(e)
            retryable = any(s in msg for s in ("UNAVAILABLE", "INTERNAL")) or (
                "unrecoverable" in msg.lower()
            )
            if not retryable:
                raise
            time.sleep(75)
            _ensure_devices()
    else:
        raise last_err
    q = np.concatenate([res.results[c]["q"] for c in range(N_CORES)], axis=0)
    return np.ascontiguousarray(q.astype(np.float32)), res


def kernel(**inputs) -> np.ndarray:
    q, _ = run(trace=False, **inputs)
    return q
